# revision 37
# baseline (speedup 1.0000x reference)
"""Local (sparse) attention layer on 8 Trainium2 NeuronCores.

Sharding: core c handles batch b = c//2, query half c%2 (1024 queries),
full context of its batch (data parallel on the small Dense weights).

v6 "fully dense, zero-gather" pipeline (per core):
  The 32-neighbor sparse attention is recast as dense attention against
  the full 2048-token context, masked by a host-built neighbor-COUNT
  matrix (exactly preserving duplicate-index multiplicity):

      out_q = (sum_c cnt[c,q] exp(s[c,q]) V[c]) / (sum_c cnt[c,q] exp(s[c,q]))

  This trades the 64MB/core DMA row-gather of the v4 kernel (~186us at
  the modeled 360B/ns) for dense PE matmuls plus one dense exp pass on
  Act (the bottleneck: 128 x 1038ns back-to-back).

  Per core:
    A. PE projections from host-transposed activations; per-head
       qT_h [64,1024] / kT_h [64,2048] tiles (matmul operands must start
       at partition 0 on this device), V [2048,512] c-major. PSUM->SBUF
       copies split across DVE and Act (Act also folds the q bias via
       Identity+bias). Wq/bq pre-scaled by 1/sqrt(hd) on the host.
    B. Per (query tile t, context block cb) unit:
       - PE: S^T[c, (h,q)] = kT_h^T qT_h  (8 matmuls, f32 PSUM)
       - Act: A = exp(S^T) -> SBUF bf16
       - DVE: A *= cnt[c,q] (broadcast over heads)
       - PE: av[q,(h,d)] += A_h^T V_h ; den[q,h] += A_h^T ones
         (single PSUM accumulation group per bank across all 128 matmuls)
    C. Tail per tile, split so PSUM frees fast and PE never head-of-line
       blocks: tail1 (DVE: 1/den, av->SBUF) runs immediately; tail2
       (Pool normalize, PE transpose + out-projection, bias, DMA out)
       is deferred a few units into the next tile.
"""

import numpy as np

HEADS = 8
HD = 64
DIM = 512
DIN = 256
B, N, M, K = 4, 2048, 2048, 32
N_LOC = 1024  # queries per core
NT = N_LOC // 128  # query tiles per core
NCB = M // 128  # context blocks

_CACHE = {}


def _build():
    import concourse.bass as bass
    import concourse.bacc as bacc
    import concourse.mybir as mybir
    from concourse.tile import TileContext
    from concourse.masks import make_identity

    f32 = mybir.dt.float32
    bf16 = mybir.dt.bfloat16
    Act = mybir.ActivationFunctionType

    nc = bacc.Bacc("TRN2")
    xT_h = nc.dram_tensor("xT", [DIN, N_LOC], bf16, kind="ExternalInput")
    cT_h = nc.dram_tensor("cT", [DIN, M], bf16, kind="ExternalInput")
    cnt_h = nc.dram_tensor("cnt", [128, NT * NCB * 128], bf16, kind="ExternalInput")
    wqkv_h = nc.dram_tensor("wqkv", [DIN, 3 * DIM], bf16, kind="ExternalInput")
    wo_h = nc.dram_tensor("wo", [DIM, DIN], bf16, kind="ExternalInput")
    bq_h = nc.dram_tensor("bq", [128, 4], f32, kind="ExternalInput")
    bo_h = nc.dram_tensor("bo", [128, DIN], f32, kind="ExternalInput")
    out_h = nc.dram_tensor("out", [N_LOC, DIN], f32, kind="ExternalOutput")

    with TileContext(nc) as tc:
        with tc.tile_pool(name="const", bufs=1) as cpool:
            ident = cpool.tile([128, 128], bf16)
            make_identity(nc, ident[:])
            ones_sb = cpool.tile([128, 1], bf16)
            nc.vector.memset(ones_sb[:], 1.0)
            w_sb = [cpool.tile([128, 3 * DIM], bf16, tag=f"w{c}", name=f"w{c}") for c in range(2)]
            wo_sb = cpool.tile([128, 4 * DIN], bf16)
            bqc_sb = cpool.tile([128, 4], f32)
            bo_sb = cpool.tile([128, DIN], f32)
            for c in range(2):
                nc.sync.dma_start(out=w_sb[c][:], in_=wqkv_h[c * 128:(c + 1) * 128, :])
            # wo [512, 256] -> [128, (chunk, 256)] in one 3D-AP DMA
            nc.sync.dma_start(
                out=wo_sb[:].rearrange("p (c j) -> p c j", c=4),
                in_=wo_h[:].rearrange("(c p) j -> p c j", c=4))
            nc.sync.dma_start(out=bqc_sb[:], in_=bq_h[:])
            nc.sync.dma_start(out=bo_sb[:], in_=bo_h[:])

            def wq(c, r):
                return w_sb[c][:, r * 128:(r + 1) * 128]

            def wk(c, r):
                return w_sb[c][:, DIM + r * 128:DIM + (r + 1) * 128]

            def wv(c):
                return w_sb[c][:, 2 * DIM:3 * DIM]

            with tc.tile_pool(name="perm", bufs=1) as ppool:
                # head-pair projection tiles [128, .] (head 2r at partitions
                # 0-63, head 2r+1 at 64-127) plus [64, .] odd-head tiles
                # filled by SBUF->SBUF DMA partition shuffles: matmul
                # operands must start at partition 0 on this device, so the
                # odd heads need their own partition-0-based tiles.
                qT_pr = [ppool.tile([128, N_LOC], bf16, tag=f"qP{r}", name=f"qP{r}") for r in range(4)]
                kT_pr = [ppool.tile([128, M], bf16, tag=f"kP{r}", name=f"kP{r}") for r in range(4)]
                qT_od = [ppool.tile([64, N_LOC], bf16, tag=f"qO{r}", name=f"qO{r}") for r in range(4)]
                kT_od = [ppool.tile([64, M], bf16, tag=f"kO{r}", name=f"kO{r}") for r in range(4)]
                v_sb = [ppool.tile([128, DIM], bf16, tag=f"v{cb}", name=f"v{cb}") for cb in range(NCB)]

                def qT(h):
                    return qT_pr[h // 2][0:64, :] if h % 2 == 0 else qT_od[h // 2][:]

                def kT(h):
                    return kT_pr[h // 2][0:64, :] if h % 2 == 0 else kT_od[h // 2][:]

                # single rotating PSUM pool (tag "s": [128,1024] f32, 2
                # banks x 3 bufs) shared by phase-A projections, S^T units
                # and tail transposes/out-proj: avoids the pool-scope WAR
                # that would serialize phase A against the first units.
                with (
                    tc.tile_pool(name="inp", bufs=1) as ipool,
                    tc.tile_pool(name="cntp", bufs=2) as cntp,
                    tc.tile_pool(name="atp", bufs=10) as atp,
                    tc.tile_pool(name="tailp", bufs=2) as tailp,
                    tc.tile_pool(name="psS", bufs=3, space="PSUM") as psS,
                    tc.tile_pool(name="psav", bufs=1, space="PSUM") as psav,
                    tc.tile_pool(name="psden", bufs=1, space="PSUM") as psden,
                ):
                    # ---- phase A: projections (pair-packed) ----
                    xT_sb = [ipool.tile([128, N_LOC], bf16, tag=f"xT{c}", name=f"xT{c}") for c in range(2)]
                    cT_sb = [ipool.tile([128, M], bf16, tag=f"cT{c}", name=f"cT{c}") for c in range(2)]
                    for c in range(2):
                        nc.scalar.dma_start(out=xT_sb[c][:], in_=xT_h[c * 128:(c + 1) * 128, :])
                        nc.scalar.dma_start(out=cT_sb[c][:], in_=cT_h[c * 128:(c + 1) * 128, :])
                    # phase-A psum: pack TWO projection chunks per s-slot
                    # (separate banks, separate accumulation groups) so PE
                    # gets enough runway to ramp out of the mid p-state
                    pa_state = {"tile": None, "n": 0}

                    def pa_half():
                        if pa_state["tile"] is not None and pa_state["n"] == 1:
                            pa_state["n"] = 2
                            return pa_state["tile"][:, 512:1024]
                        pa_state["tile"] = psS.tile([128, 1024], f32, tag="s", name="pa")
                        pa_state["n"] = 1
                        return pa_state["tile"][:, 0:512]

                    # kT pair r: [128, c] = Wk[:, pair]^T ctx^T; each chunk
                    # is followed by the odd head's partition-0 shuffle DMA
                    def k_proj(r, cc):
                        psk = pa_half()
                        for c in range(2):
                            nc.tensor.matmul(
                                out=psk, lhsT=wk(c, r),
                                rhs=cT_sb[c][:, cc * 512:(cc + 1) * 512],
                                start=(c == 0), stop=(c == 1))
                        dst = kT_pr[r][:, cc * 512:(cc + 1) * 512]
                        if cc % 2 == 0:
                            nc.vector.tensor_copy(out=dst, in_=psk)
                        else:
                            nc.scalar.activation(out=dst, in_=psk, func=Act.Copy)
                        if cc == 3:
                            nc.sync.dma_start(out=kT_od[r][:], in_=kT_pr[r][64:128, :])

                    def q_proj(r, cc):
                        psq = pa_half()
                        for c in range(2):
                            nc.tensor.matmul(
                                out=psq, lhsT=wq(c, r),
                                rhs=xT_sb[c][:, cc * 512:(cc + 1) * 512],
                                start=(c == 0), stop=(c == 1))
                        nc.scalar.activation(
                            out=qT_pr[r][:, cc * 512:(cc + 1) * 512],
                            in_=psq, func=Act.Identity,
                            bias=bqc_sb[:, r:r + 1])
                        if cc == 1:
                            nc.sync.dma_start(out=qT_od[r][:], in_=qT_pr[r][64:128, :])

                    # upfront: all of k and q (interleaving them into the
                    # loop trades lead-in 1:1 for in-loop stalls); V blocks
                    # 8-15 ride the first tile's slack
                    for r in range(4):
                        for cc in range(4):
                            k_proj(r, cc)
                    for r in range(4):
                        q_proj(r, 0)
                        q_proj(r, 1)

                    def v_proj(cb, eng):
                        psv = pa_half()
                        for c in range(2):
                            nc.tensor.matmul(
                                out=psv,
                                lhsT=cT_sb[c][:, cb * 128:(cb + 1) * 128],
                                rhs=wv(c),
                                start=(c == 0), stop=(c == 1))
                        with nc.allow_low_precision(reason="bf16 V"):
                            eng.tensor_copy(out=v_sb[cb][:], in_=psv)

                    # only the first blocks up front; the rest interleaves
                    # with the first tile's units (gpsimd copies: DVE is busy)
                    for cb0 in range(8):
                        v_proj(cb0, nc.vector)

                    # ---- phase B: dense attention ----
                    pend = []  # deferred tail closures [(norm, pe_part)]

                    def mk_tail2(t, avf, rden):
                        ao = tailp.tile([128, DIM], bf16, tag="ao", name=f"ao{t}")

                        def norm():
                            with nc.allow_low_precision(reason="bf16 attention out"):
                                # inner-dim (d) stride-0 broadcast: gpsimd (DVE
                                # lacks it on hw; gpsimd can't read PSUM)
                                nc.gpsimd.tensor_tensor(
                                    out=ao[:].rearrange("p (h d) -> p h d", h=8),
                                    in0=avf[:].rearrange("p (h d) -> p h d", h=8),
                                    in1=rden[:].rearrange("p (h o) -> p h o", o=1
                                                          ).to_broadcast([128, 8, 64]),
                                    op=mybir.AluOpType.mult)

                        aT2 = tailp.tile([128, DIM], bf16, tag="aT2", name=f"aT2{t}")

                        def pe_t():
                            # holds one s-slot for < 1 Act period
                            ptile = psS.tile([128, 1024], f32, tag="s", name=f"pt{t}")
                            pst = ptile[:].bitcast(bf16)[:, 0:DIM]
                            for c in range(4):
                                nc.tensor.transpose(
                                    out=pst[:, c * 128:(c + 1) * 128],
                                    in_=ao[:, c * 128:(c + 1) * 128],
                                    identity=ident[:])
                            nc.vector.tensor_copy(out=aT2[:], in_=pst)

                        def pe_o():
                            ptile = psS.tile([128, 1024], f32, tag="s", name=f"po{t}")
                            op_ps = ptile[:, 0:DIN]
                            for c in range(4):
                                nc.tensor.matmul(
                                    out=op_ps, lhsT=aT2[:, c * 128:(c + 1) * 128],
                                    rhs=wo_sb[:, c * DIN:(c + 1) * DIN],
                                    start=(c == 0), stop=(c == 3))
                            ot = tailp.tile([128, DIN], f32, tag="ot", name=f"ot{t}")
                            nc.vector.tensor_tensor(
                                out=ot[:], in0=op_ps, in1=bo_sb[:],
                                op=mybir.AluOpType.add)
                            nc.sync.dma_start(
                                out=out_h[t * 128:(t + 1) * 128, :], in_=ot[:])

                        return norm, pe_t, pe_o

                    # prefetch the first tile's cnt before the units
                    cnt_tiles = {}
                    cnt_tiles[0] = cntp.tile([128, NCB * 128], bf16, tag="cnt", name="cnt0")
                    nc.sync.dma_start(out=cnt_tiles[0][:], in_=cnt_h[:, 0:2048])

                    from collections import deque
                    av_q = deque()
                    for t in range(NT):
                        cnt_sb = cnt_tiles.pop(t)
                        av_ps = psav.tile([128, DIM], f32, tag="av", name=f"av{t}")
                        den_ps = psden.tile([128, 8], f32, tag="den", name=f"den{t}")
                        for cb in range(NCB):
                            # flush the previous tile's deferred tail work:
                            # Pool-normalize early, PE transposes/out-proj
                            # later (so PE never head-of-line blocks on them)
                            if cb == 1 and pend:
                                pend[-1][0]()
                            if cb == 4 and pend:
                                pend[-1][1]()
                            if cb == 7 and pend:
                                pend.pop()[2]()
                            # deadline-scheduled phase-A remainder rides
                            # the first tiles' slack
                            if t == 0 and cb % 2 == 0:
                                v_proj(8 + cb // 2, nc.vector)  # V 8-15, need cb 8-15
                            # prefetch next tile's cnt mid-tile
                            if cb == 8 and t + 1 < NT:
                                cnt_tiles[t + 1] = cntp.tile(
                                    [128, NCB * 128], bf16, tag="cnt", name=f"cnt{t + 1}")
                                nc.sync.dma_start(
                                    out=cnt_tiles[t + 1][:],
                                    in_=cnt_h[:, (t + 1) * 2048:(t + 2) * 2048])
                            # S^T[c, (h,q)] for this (t, cb)
                            s_ps = psS.tile([128, 1024], f32, tag="s", name=f"s{t}_{cb}")
                            for h in range(8):
                                nc.tensor.matmul(
                                    out=s_ps[:, h * 128:(h + 1) * 128],
                                    lhsT=kT(h)[:, cb * 128:(cb + 1) * 128],
                                    rhs=qT(h)[:, t * 128:(t + 1) * 128],
                                    start=True, stop=True)
                            # A = exp(S^T) (Act), then A *= cnt (DVE)
                            slab = atp.tile([128, 8, 128], bf16, tag="at", name=f"at{t}_{cb}")
                            nc.scalar.activation(
                                out=slab[:].rearrange("p h q -> p (h q)"),
                                in_=s_ps[:], func=Act.Exp)
                            with nc.allow_low_precision(reason="bf16 attention weights"):
                                nc.vector.tensor_tensor(
                                    out=slab[:], in0=slab[:],
                                    in1=cnt_sb[:, cb * 128:(cb + 1) * 128].rearrange(
                                        "p (o q) -> p o q", o=1
                                    ).to_broadcast([128, 8, 128]),
                                    op=mybir.AluOpType.mult)
                            # av[q, (h,d)] += A_h^T V_h ; den[q, h] += A_h^T 1
                            # one PSUM accumulation group per bank: start on
                            # the first matmul, stop on the last. Emission is
                            # deferred 2 units so these matmuls enter the PE
                            # queue with deps resolved (a stalled av chain
                            # fills the 4-deep PE wait queue and blocks the
                            # next unit's S matmuls).
                            def av_mms(cb=cb, slab=slab, av_ps=av_ps, den_ps=den_ps):
                                for h in range(8):
                                    nc.tensor.matmul(
                                        out=av_ps[:, h * 64:(h + 1) * 64],
                                        lhsT=slab[:, h, :],
                                        rhs=v_sb[cb][:, h * 64:(h + 1) * 64],
                                        start=(cb == 0 and h == 0),
                                        stop=(cb == NCB - 1 and h == 7))
                                    nc.tensor.matmul(
                                        out=den_ps[:, h:h + 1],
                                        lhsT=slab[:, h, :],
                                        rhs=ones_sb[:],
                                        start=(cb == 0 and h == 0),
                                        stop=(cb == NCB - 1 and h == 7))
                            av_q.append(av_mms)
                            keep = 2 if cb < 8 else 0
                            while len(av_q) > keep:
                                av_q.popleft()()
                        while av_q:
                            av_q.popleft()()
                        # tail1: free av/den PSUM fast (DVE only)
                        rden = tailp.tile([128, 8], f32, tag="rden", name=f"rden{t}")
                        nc.vector.reciprocal(out=rden[:], in_=den_ps[:])
                        avf = tailp.tile([128, DIM], f32, tag="avf", name=f"avf{t}")
                        nc.vector.tensor_copy(out=avf[:], in_=av_ps[:])
                        pend.append(mk_tail2(t, avf, rden))
                    norm, pe_t, pe_o = pend.pop()
                    norm()
                    pe_t()
                    pe_o()
    nc.compile()
    return nc


def _get_nc():
    if "nc" not in _CACHE:
        _CACHE["nc"] = _build()
    return _CACHE["nc"]


def kernel(**inputs) -> np.ndarray:
    from concourse.bass_utils import run_bass_kernel_spmd
    from ml_dtypes import bfloat16

    x = np.asarray(inputs["x"], dtype=np.float32)
    ctx = np.asarray(inputs["context"], dtype=np.float32)
    idx = np.asarray(inputs["index_pairs"]).astype(np.int64)
    scale = 1.0 / np.sqrt(HD)
    wq = np.asarray(inputs["Wq"], dtype=np.float32) * scale
    wqkv = np.concatenate(
        [wq, np.asarray(inputs["Wk"], dtype=np.float32),
         np.asarray(inputs["Wv"], dtype=np.float32)], axis=1).astype(bfloat16)
    bq = (np.asarray(inputs["bq"], dtype=np.float32) * scale).reshape(4, 128).T
    bq = np.ascontiguousarray(bq).astype(np.float32)  # [128, 4], col r = bq[r*128:(r+1)*128]
    wo = np.asarray(inputs["Wout"], dtype=np.float32).astype(bfloat16)
    bo = np.tile(np.asarray(inputs["bout"], dtype=np.float32).reshape(1, DIN),
                 (128, 1)).astype(np.float32)

    nc = _get_nc()
    in_maps = []
    qrep = np.repeat(np.arange(N_LOC), K)
    for c in range(8):
        b, half = c // 2, c % 2
        xT_c = np.ascontiguousarray(
            x[b, half * N_LOC:(half + 1) * N_LOC, :].T).astype(bfloat16)
        cT_c = np.ascontiguousarray(ctx[b].T).astype(bfloat16)
        idx_c = idx[b, half * N_LOC:(half + 1) * N_LOC, :]  # [1024, 32]
        # neighbor count matrix cnt[c, q], including duplicate multiplicity
        flat = idx_c.reshape(-1) * N_LOC + qrep
        cnt = np.bincount(flat, minlength=M * N_LOC).reshape(M, N_LOC)
        # layout [128 c-part, (t, cb, q)]
        cnt_w = np.ascontiguousarray(
            cnt.reshape(NCB, 128, NT, 128).transpose(1, 2, 0, 3).reshape(128, NT * NCB * 128)
        ).astype(bfloat16)
        in_maps.append({
            "xT": xT_c, "cT": cT_c, "cnt": cnt_w,
            "wqkv": wqkv, "wo": wo, "bq": bq, "bo": bo,
        })
    res = run_bass_kernel_spmd(nc, in_maps, core_ids=list(range(8)))
    out = np.empty((B, N, DIN), dtype=np.float32)
    for c in range(8):
        b, half = c // 2, c % 2
        out[b, half * N_LOC:(half + 1) * N_LOC, :] = res.results[c]["out"]
    return out


# revision 41
# speedup vs baseline: 1.0041x; 1.0041x over previous
"""Local (sparse) attention layer on 8 Trainium2 NeuronCores.

Sharding: core c handles batch b = c//2, query half c%2 (1024 queries),
full context of its batch (data parallel on the small Dense weights).

v6 "fully dense, zero-gather" pipeline (per core):
  The 32-neighbor sparse attention is recast as dense attention against
  the full 2048-token context, masked by a host-built neighbor-COUNT
  matrix (exactly preserving duplicate-index multiplicity):

      out_q = (sum_c cnt[c,q] exp(s[c,q]) V[c]) / (sum_c cnt[c,q] exp(s[c,q]))

  This trades the 64MB/core DMA row-gather of the v4 kernel (~186us at
  the modeled 360B/ns) for dense PE matmuls plus one dense exp pass on
  Act (the bottleneck: 128 x 1038ns back-to-back).

  Per core:
    A. PE projections from host-transposed activations; per-head
       qT_h [64,1024] / kT_h [64,2048] tiles (matmul operands must start
       at partition 0 on this device), V [2048,512] c-major. PSUM->SBUF
       copies split across DVE and Act (Act also folds the q bias via
       Identity+bias). Wq/bq pre-scaled by 1/sqrt(hd) on the host.
    B. Per (query tile t, context block cb) unit:
       - PE: S^T[c, (h,q)] = kT_h^T qT_h  (8 matmuls, f32 PSUM)
       - Act: A = exp(S^T) -> SBUF bf16
       - DVE: A *= cnt[c,q] (broadcast over heads)
       - PE: av[q,(h,d)] += A_h^T V_h ; den[q,h] += A_h^T ones
         (single PSUM accumulation group per bank across all 128 matmuls)
    C. Tail per tile, split so PSUM frees fast and PE never head-of-line
       blocks: tail1 (DVE: 1/den, av->SBUF) runs immediately; tail2
       (Pool normalize, PE transpose + out-projection, bias, DMA out)
       is deferred a few units into the next tile.
"""

import numpy as np

HEADS = 8
HD = 64
DIM = 512
DIN = 256
B, N, M, K = 4, 2048, 2048, 32
N_LOC = 1024  # queries per core
NT = N_LOC // 128  # query tiles per core
NCB = M // 128  # context blocks

_CACHE = {}


def _build():
    import concourse.bass as bass
    import concourse.bacc as bacc
    import concourse.mybir as mybir
    from concourse.tile import TileContext
    from concourse.masks import make_identity

    f32 = mybir.dt.float32
    bf16 = mybir.dt.bfloat16
    Act = mybir.ActivationFunctionType

    nc = bacc.Bacc("TRN2")
    xT_h = nc.dram_tensor("xT", [DIN, N_LOC], bf16, kind="ExternalInput")
    cT_h = nc.dram_tensor("cT", [DIN, M], bf16, kind="ExternalInput")
    cnt_h = nc.dram_tensor("cnt", [128, NT * NCB * 128], bf16, kind="ExternalInput")
    wqkv_h = nc.dram_tensor("wqkv", [DIN, 3 * DIM], bf16, kind="ExternalInput")
    wo_h = nc.dram_tensor("wo", [DIM, DIN], bf16, kind="ExternalInput")
    bq_h = nc.dram_tensor("bq", [128, 4], f32, kind="ExternalInput")
    bo_h = nc.dram_tensor("bo", [128, DIN], f32, kind="ExternalInput")
    out_h = nc.dram_tensor("out", [N_LOC, DIN], f32, kind="ExternalOutput")

    with TileContext(nc) as tc:
        with tc.tile_pool(name="const", bufs=1) as cpool:
            ident = cpool.tile([128, 128], bf16)
            make_identity(nc, ident[:])
            ones_sb = cpool.tile([128, 1], bf16)
            nc.vector.memset(ones_sb[:], 1.0)
            w_sb = [cpool.tile([128, 3 * DIM], bf16, tag=f"w{c}", name=f"w{c}") for c in range(2)]
            wo_sb = cpool.tile([128, 4 * DIN], bf16)
            bqc_sb = cpool.tile([128, 4], f32)
            bo_sb = cpool.tile([128, DIN], f32)
            for c in range(2):
                nc.sync.dma_start(out=w_sb[c][:], in_=wqkv_h[c * 128:(c + 1) * 128, :])
            # wo [512, 256] -> [128, (chunk, 256)] in one 3D-AP DMA
            nc.sync.dma_start(
                out=wo_sb[:].rearrange("p (c j) -> p c j", c=4),
                in_=wo_h[:].rearrange("(c p) j -> p c j", c=4))
            nc.sync.dma_start(out=bqc_sb[:], in_=bq_h[:])
            nc.sync.dma_start(out=bo_sb[:], in_=bo_h[:])

            def wq(c, r):
                return w_sb[c][:, r * 128:(r + 1) * 128]

            def wk(c, r):
                return w_sb[c][:, DIM + r * 128:DIM + (r + 1) * 128]

            def wv(c):
                return w_sb[c][:, 2 * DIM:3 * DIM]

            with tc.tile_pool(name="perm", bufs=1) as ppool:
                # head-pair projection tiles [128, .] (head 2r at partitions
                # 0-63, head 2r+1 at 64-127) plus [64, .] odd-head tiles
                # filled by SBUF->SBUF DMA partition shuffles: matmul
                # operands must start at partition 0 on this device, so the
                # odd heads need their own partition-0-based tiles.
                qT_pr = [ppool.tile([128, N_LOC], bf16, tag=f"qP{r}", name=f"qP{r}") for r in range(4)]
                kT_pr = [ppool.tile([128, M], bf16, tag=f"kP{r}", name=f"kP{r}") for r in range(4)]
                qT_od = [ppool.tile([64, N_LOC], bf16, tag=f"qO{r}", name=f"qO{r}") for r in range(4)]
                kT_od = [ppool.tile([64, M], bf16, tag=f"kO{r}", name=f"kO{r}") for r in range(4)]
                v_sb = [ppool.tile([128, DIM], bf16, tag=f"v{cb}", name=f"v{cb}") for cb in range(NCB)]

                def qT(h):
                    return qT_pr[h // 2][0:64, :] if h % 2 == 0 else qT_od[h // 2][:]

                def kT(h):
                    return kT_pr[h // 2][0:64, :] if h % 2 == 0 else kT_od[h // 2][:]

                # single rotating PSUM pool (tag "s": [128,1024] f32, 2
                # banks x 3 bufs) shared by phase-A projections, S^T units
                # and tail transposes/out-proj: avoids the pool-scope WAR
                # that would serialize phase A against the first units.
                with (
                    tc.tile_pool(name="inp", bufs=1) as ipool,
                    tc.tile_pool(name="cntp", bufs=2) as cntp,
                    tc.tile_pool(name="atp", bufs=10) as atp,
                    tc.tile_pool(name="tailp", bufs=2) as tailp,
                    tc.tile_pool(name="psS", bufs=3, space="PSUM") as psS,
                    tc.tile_pool(name="psav", bufs=1, space="PSUM") as psav,
                    tc.tile_pool(name="psden", bufs=1, space="PSUM") as psden,
                ):
                    # ---- phase A: projections (pair-packed) ----
                    xT_sb = [ipool.tile([128, N_LOC], bf16, tag=f"xT{c}", name=f"xT{c}") for c in range(2)]
                    cT_sb = [ipool.tile([128, M], bf16, tag=f"cT{c}", name=f"cT{c}") for c in range(2)]
                    for c in range(2):
                        nc.scalar.dma_start(out=xT_sb[c][:], in_=xT_h[c * 128:(c + 1) * 128, :])
                        nc.scalar.dma_start(out=cT_sb[c][:], in_=cT_h[c * 128:(c + 1) * 128, :])
                    # phase-A psum: pack TWO projection chunks per s-slot
                    # (separate banks, separate accumulation groups) so PE
                    # gets enough runway to ramp out of the mid p-state
                    pa_state = {"tile": None, "n": 0}

                    def pa_half():
                        if pa_state["tile"] is not None and pa_state["n"] == 1:
                            pa_state["n"] = 2
                            return pa_state["tile"][:, 512:1024]
                        pa_state["tile"] = psS.tile([128, 1024], f32, tag="s", name="pa")
                        pa_state["n"] = 1
                        return pa_state["tile"][:, 0:512]

                    # kT pair r: [128, c] = Wk[:, pair]^T ctx^T; each chunk
                    # is followed by the odd head's partition-0 shuffle DMA
                    def k_proj(r, cc):
                        psk = pa_half()
                        for c in range(2):
                            nc.tensor.matmul(
                                out=psk, lhsT=wk(c, r),
                                rhs=cT_sb[c][:, cc * 512:(cc + 1) * 512],
                                start=(c == 0), stop=(c == 1))
                        dst = kT_pr[r][:, cc * 512:(cc + 1) * 512]
                        if cc % 2 == 0:
                            nc.vector.tensor_copy(out=dst, in_=psk)
                        else:
                            nc.scalar.activation(out=dst, in_=psk, func=Act.Copy)
                        nc.sync.dma_start(
                            out=kT_od[r][:, cc * 512:(cc + 1) * 512],
                            in_=kT_pr[r][64:128, cc * 512:(cc + 1) * 512])

                    def q_proj(r, cc):
                        psq = pa_half()
                        for c in range(2):
                            nc.tensor.matmul(
                                out=psq, lhsT=wq(c, r),
                                rhs=xT_sb[c][:, cc * 512:(cc + 1) * 512],
                                start=(c == 0), stop=(c == 1))
                        nc.scalar.activation(
                            out=qT_pr[r][:, cc * 512:(cc + 1) * 512],
                            in_=psq, func=Act.Identity,
                            bias=bqc_sb[:, r:r + 1])
                        nc.sync.dma_start(
                            out=qT_od[r][:, cc * 512:(cc + 1) * 512],
                            in_=qT_pr[r][64:128, cc * 512:(cc + 1) * 512])

                    # upfront: only what units cb0-3 need; the rest is
                    # deadline-scheduled into tile 0 in packed pairs (each
                    # pair holds one s-slot for < 2 Act periods)
                    for r in range(4):
                        k_proj(r, 0)
                    for r in range(4):
                        q_proj(r, 0)
                    for cb0 in range(4):
                        v_proj(cb0, nc.vector)
                    sched = {
                        0: [("k", 0, 1), ("k", 1, 1)],
                        1: [("k", 2, 1), ("k", 3, 1)],
                        2: [("v", 4), ("v", 5)],
                        3: [("v", 6), ("v", 7)],
                        4: [("k", 0, 2), ("k", 1, 2)],
                        5: [("k", 2, 2), ("k", 3, 2)],
                        6: [("v", 8), ("v", 9)],
                        7: [("v", 10), ("v", 11)],
                        8: [("k", 0, 3), ("k", 1, 3)],
                        9: [("k", 2, 3), ("k", 3, 3)],
                        10: [("v", 12), ("v", 13)],
                        11: [("v", 14), ("v", 15)],
                        12: [("q", 0, 1), ("q", 1, 1)],
                        13: [("q", 2, 1), ("q", 3, 1)],
                    }

                    def v_proj(cb, eng):
                        psv = pa_half()
                        for c in range(2):
                            nc.tensor.matmul(
                                out=psv,
                                lhsT=cT_sb[c][:, cb * 128:(cb + 1) * 128],
                                rhs=wv(c),
                                start=(c == 0), stop=(c == 1))
                        with nc.allow_low_precision(reason="bf16 V"):
                            eng.tensor_copy(out=v_sb[cb][:], in_=psv)

                    # ---- phase B: dense attention ----
                    pend = []  # deferred tail closures [(norm, pe_part)]

                    def mk_tail2(t, avf, rden):
                        ao = tailp.tile([128, DIM], bf16, tag="ao", name=f"ao{t}")

                        def norm():
                            with nc.allow_low_precision(reason="bf16 attention out"):
                                # inner-dim (d) stride-0 broadcast: gpsimd (DVE
                                # lacks it on hw; gpsimd can't read PSUM)
                                nc.gpsimd.tensor_tensor(
                                    out=ao[:].rearrange("p (h d) -> p h d", h=8),
                                    in0=avf[:].rearrange("p (h d) -> p h d", h=8),
                                    in1=rden[:].rearrange("p (h o) -> p h o", o=1
                                                          ).to_broadcast([128, 8, 64]),
                                    op=mybir.AluOpType.mult)

                        aT2 = tailp.tile([128, DIM], bf16, tag="aT2", name=f"aT2{t}")

                        def pe_t():
                            # holds one s-slot for < 1 Act period
                            ptile = psS.tile([128, 1024], f32, tag="s", name=f"pt{t}")
                            pst = ptile[:].bitcast(bf16)[:, 0:DIM]
                            for c in range(4):
                                nc.tensor.transpose(
                                    out=pst[:, c * 128:(c + 1) * 128],
                                    in_=ao[:, c * 128:(c + 1) * 128],
                                    identity=ident[:])
                            nc.vector.tensor_copy(out=aT2[:], in_=pst)

                        def pe_o():
                            ptile = psS.tile([128, 1024], f32, tag="s", name=f"po{t}")
                            op_ps = ptile[:, 0:DIN]
                            for c in range(4):
                                nc.tensor.matmul(
                                    out=op_ps, lhsT=aT2[:, c * 128:(c + 1) * 128],
                                    rhs=wo_sb[:, c * DIN:(c + 1) * DIN],
                                    start=(c == 0), stop=(c == 3))
                            ot = tailp.tile([128, DIN], f32, tag="ot", name=f"ot{t}")
                            nc.vector.tensor_tensor(
                                out=ot[:], in0=op_ps, in1=bo_sb[:],
                                op=mybir.AluOpType.add)
                            nc.sync.dma_start(
                                out=out_h[t * 128:(t + 1) * 128, :], in_=ot[:])

                        return norm, pe_t, pe_o

                    # prefetch the first tile's cnt before the units
                    cnt_tiles = {}
                    cnt_tiles[0] = cntp.tile([128, NCB * 128], bf16, tag="cnt", name="cnt0")
                    nc.sync.dma_start(out=cnt_tiles[0][:], in_=cnt_h[:, 0:2048])

                    from collections import deque
                    av_q = deque()
                    for t in range(NT):
                        cnt_sb = cnt_tiles.pop(t)
                        av_ps = psav.tile([128, DIM], f32, tag="av", name=f"av{t}")
                        den_ps = psden.tile([128, 8], f32, tag="den", name=f"den{t}")
                        for cb in range(NCB):
                            # flush the previous tile's deferred tail work:
                            # Pool-normalize early, PE transposes/out-proj
                            # later (so PE never head-of-line blocks on them)
                            if cb == 1 and pend:
                                pend[-1][0]()
                            if cb == 4 and pend:
                                pend[-1][1]()
                            if cb == 7 and pend:
                                pend.pop()[2]()
                            # deadline-scheduled phase-A remainder rides
                            # the first tiles' slack
                            if t == 0:
                                for item in sched.pop(cb, []):
                                    if item[0] == "k":
                                        k_proj(item[1], item[2])
                                    elif item[0] == "q":
                                        q_proj(item[1], item[2])
                                    else:
                                        v_proj(item[1], nc.vector)
                            # prefetch next tile's cnt mid-tile
                            if cb == 8 and t + 1 < NT:
                                cnt_tiles[t + 1] = cntp.tile(
                                    [128, NCB * 128], bf16, tag="cnt", name=f"cnt{t + 1}")
                                nc.sync.dma_start(
                                    out=cnt_tiles[t + 1][:],
                                    in_=cnt_h[:, (t + 1) * 2048:(t + 2) * 2048])
                            # S^T[c, (h,q)] for this (t, cb)
                            s_ps = psS.tile([128, 1024], f32, tag="s", name=f"s{t}_{cb}")
                            for h in range(8):
                                nc.tensor.matmul(
                                    out=s_ps[:, h * 128:(h + 1) * 128],
                                    lhsT=kT(h)[:, cb * 128:(cb + 1) * 128],
                                    rhs=qT(h)[:, t * 128:(t + 1) * 128],
                                    start=True, stop=True)
                            # A = exp(S^T) (Act), then A *= cnt (DVE)
                            slab = atp.tile([128, 8, 128], bf16, tag="at", name=f"at{t}_{cb}")
                            nc.scalar.activation(
                                out=slab[:].rearrange("p h q -> p (h q)"),
                                in_=s_ps[:], func=Act.Exp)
                            with nc.allow_low_precision(reason="bf16 attention weights"):
                                nc.vector.tensor_tensor(
                                    out=slab[:], in0=slab[:],
                                    in1=cnt_sb[:, cb * 128:(cb + 1) * 128].rearrange(
                                        "p (o q) -> p o q", o=1
                                    ).to_broadcast([128, 8, 128]),
                                    op=mybir.AluOpType.mult)
                            # av[q, (h,d)] += A_h^T V_h ; den[q, h] += A_h^T 1
                            # one PSUM accumulation group per bank: start on
                            # the first matmul, stop on the last. Emission is
                            # deferred 2 units so these matmuls enter the PE
                            # queue with deps resolved (a stalled av chain
                            # fills the 4-deep PE wait queue and blocks the
                            # next unit's S matmuls).
                            def av_mms(cb=cb, slab=slab, av_ps=av_ps, den_ps=den_ps):
                                for h in range(8):
                                    nc.tensor.matmul(
                                        out=av_ps[:, h * 64:(h + 1) * 64],
                                        lhsT=slab[:, h, :],
                                        rhs=v_sb[cb][:, h * 64:(h + 1) * 64],
                                        start=(cb == 0 and h == 0),
                                        stop=(cb == NCB - 1 and h == 7))
                                    nc.tensor.matmul(
                                        out=den_ps[:, h:h + 1],
                                        lhsT=slab[:, h, :],
                                        rhs=ones_sb[:],
                                        start=(cb == 0 and h == 0),
                                        stop=(cb == NCB - 1 and h == 7))
                            av_q.append(av_mms)
                            keep = 2 if cb < 8 else 0
                            while len(av_q) > keep:
                                av_q.popleft()()
                        while av_q:
                            av_q.popleft()()
                        # tail1: free av/den PSUM fast (DVE only)
                        rden = tailp.tile([128, 8], f32, tag="rden", name=f"rden{t}")
                        nc.vector.reciprocal(out=rden[:], in_=den_ps[:])
                        avf = tailp.tile([128, DIM], f32, tag="avf", name=f"avf{t}")
                        nc.vector.tensor_copy(out=avf[:], in_=av_ps[:])
                        pend.append(mk_tail2(t, avf, rden))
                    norm, pe_t, pe_o = pend.pop()
                    norm()
                    pe_t()
                    pe_o()
    nc.compile()
    return nc


def _get_nc():
    if "nc" not in _CACHE:
        _CACHE["nc"] = _build()
    return _CACHE["nc"]


def kernel(**inputs) -> np.ndarray:
    from concourse.bass_utils import run_bass_kernel_spmd
    from ml_dtypes import bfloat16

    x = np.asarray(inputs["x"], dtype=np.float32)
    ctx = np.asarray(inputs["context"], dtype=np.float32)
    idx = np.asarray(inputs["index_pairs"]).astype(np.int64)
    scale = 1.0 / np.sqrt(HD)
    wq = np.asarray(inputs["Wq"], dtype=np.float32) * scale
    wqkv = np.concatenate(
        [wq, np.asarray(inputs["Wk"], dtype=np.float32),
         np.asarray(inputs["Wv"], dtype=np.float32)], axis=1).astype(bfloat16)
    bq = (np.asarray(inputs["bq"], dtype=np.float32) * scale).reshape(4, 128).T
    bq = np.ascontiguousarray(bq).astype(np.float32)  # [128, 4], col r = bq[r*128:(r+1)*128]
    wo = np.asarray(inputs["Wout"], dtype=np.float32).astype(bfloat16)
    bo = np.tile(np.asarray(inputs["bout"], dtype=np.float32).reshape(1, DIN),
                 (128, 1)).astype(np.float32)

    nc = _get_nc()
    in_maps = []
    qrep = np.repeat(np.arange(N_LOC), K)
    for c in range(8):
        b, half = c // 2, c % 2
        xT_c = np.ascontiguousarray(
            x[b, half * N_LOC:(half + 1) * N_LOC, :].T).astype(bfloat16)
        cT_c = np.ascontiguousarray(ctx[b].T).astype(bfloat16)
        idx_c = idx[b, half * N_LOC:(half + 1) * N_LOC, :]  # [1024, 32]
        # neighbor count matrix cnt[c, q], including duplicate multiplicity
        flat = idx_c.reshape(-1) * N_LOC + qrep
        cnt = np.bincount(flat, minlength=M * N_LOC).reshape(M, N_LOC)
        # layout [128 c-part, (t, cb, q)]
        cnt_w = np.ascontiguousarray(
            cnt.reshape(NCB, 128, NT, 128).transpose(1, 2, 0, 3).reshape(128, NT * NCB * 128)
        ).astype(bfloat16)
        in_maps.append({
            "xT": xT_c, "cT": cT_c, "cnt": cnt_w,
            "wqkv": wqkv, "wo": wo, "bq": bq, "bo": bo,
        })
    res = run_bass_kernel_spmd(nc, in_maps, core_ids=list(range(8)))
    out = np.empty((B, N, DIN), dtype=np.float32)
    for c in range(8):
        b, half = c // 2, c % 2
        out[b, half * N_LOC:(half + 1) * N_LOC, :] = res.results[c]["out"]
    return out


# revision 42
# speedup vs baseline: 1.0122x; 1.0080x over previous
"""Local (sparse) attention layer on 8 Trainium2 NeuronCores.

Sharding: core c handles batch b = c//2, query half c%2 (1024 queries),
full context of its batch (data parallel on the small Dense weights).

v6 "fully dense, zero-gather" pipeline (per core):
  The 32-neighbor sparse attention is recast as dense attention against
  the full 2048-token context, masked by a host-built neighbor-COUNT
  matrix (exactly preserving duplicate-index multiplicity):

      out_q = (sum_c cnt[c,q] exp(s[c,q]) V[c]) / (sum_c cnt[c,q] exp(s[c,q]))

  This trades the 64MB/core DMA row-gather of the v4 kernel (~186us at
  the modeled 360B/ns) for dense PE matmuls plus one dense exp pass on
  Act (the bottleneck: 128 x 1038ns back-to-back).

  Per core:
    A. PE projections from host-transposed activations; per-head
       qT_h [64,1024] / kT_h [64,2048] tiles (matmul operands must start
       at partition 0 on this device), V [2048,512] c-major. PSUM->SBUF
       copies split across DVE and Act (Act also folds the q bias via
       Identity+bias). Wq/bq pre-scaled by 1/sqrt(hd) on the host.
    B. Per (query tile t, context block cb) unit:
       - PE: S^T[c, (h,q)] = kT_h^T qT_h  (8 matmuls, f32 PSUM)
       - Act: A = exp(S^T) -> SBUF bf16
       - DVE: A *= cnt[c,q] (broadcast over heads)
       - PE: av[q,(h,d)] += A_h^T V_h ; den[q,h] += A_h^T ones
         (single PSUM accumulation group per bank across all 128 matmuls)
    C. Tail per tile, split so PSUM frees fast and PE never head-of-line
       blocks: tail1 (DVE: 1/den, av->SBUF) runs immediately; tail2
       (Pool normalize, PE transpose + out-projection, bias, DMA out)
       is deferred a few units into the next tile.
"""

import numpy as np

HEADS = 8
HD = 64
DIM = 512
DIN = 256
B, N, M, K = 4, 2048, 2048, 32
N_LOC = 1024  # queries per core
NT = N_LOC // 128  # query tiles per core
NCB = M // 128  # context blocks

_CACHE = {}


def _build():
    import concourse.bass as bass
    import concourse.bacc as bacc
    import concourse.mybir as mybir
    from concourse.tile import TileContext
    from concourse.masks import make_identity

    f32 = mybir.dt.float32
    bf16 = mybir.dt.bfloat16
    Act = mybir.ActivationFunctionType

    nc = bacc.Bacc("TRN2")
    xT_h = nc.dram_tensor("xT", [DIN, N_LOC], bf16, kind="ExternalInput")
    cT_h = nc.dram_tensor("cT", [DIN, M], bf16, kind="ExternalInput")
    cnt_h = nc.dram_tensor("cnt", [128, NT * NCB * 128], bf16, kind="ExternalInput")
    wqkv_h = nc.dram_tensor("wqkv", [DIN, 3 * DIM], bf16, kind="ExternalInput")
    wo_h = nc.dram_tensor("wo", [DIM, DIN], bf16, kind="ExternalInput")
    bq_h = nc.dram_tensor("bq", [128, 4], f32, kind="ExternalInput")
    bo_h = nc.dram_tensor("bo", [128, DIN], f32, kind="ExternalInput")
    out_h = nc.dram_tensor("out", [N_LOC, DIN], f32, kind="ExternalOutput")

    with TileContext(nc) as tc:
        with tc.tile_pool(name="const", bufs=1) as cpool:
            ident = cpool.tile([128, 128], bf16)
            make_identity(nc, ident[:])
            ones_sb = cpool.tile([128, 1], bf16)
            nc.vector.memset(ones_sb[:], 1.0)
            w_sb = [cpool.tile([128, 3 * DIM], bf16, tag=f"w{c}", name=f"w{c}") for c in range(2)]
            wo_sb = cpool.tile([128, 4 * DIN], bf16)
            bqc_sb = cpool.tile([128, 4], f32)
            bo_sb = cpool.tile([128, DIN], f32)
            for c in range(2):
                nc.sync.dma_start(out=w_sb[c][:], in_=wqkv_h[c * 128:(c + 1) * 128, :])
            nc.sync.dma_start(out=bqc_sb[:], in_=bq_h[:])

            def wq(c, r):
                return w_sb[c][:, r * 128:(r + 1) * 128]

            def wk(c, r):
                return w_sb[c][:, DIM + r * 128:DIM + (r + 1) * 128]

            def wv(c):
                return w_sb[c][:, 2 * DIM:3 * DIM]

            with tc.tile_pool(name="perm", bufs=1) as ppool:
                # head-pair projection tiles [128, .] (head 2r at partitions
                # 0-63, head 2r+1 at 64-127) plus [64, .] odd-head tiles
                # filled by SBUF->SBUF DMA partition shuffles: matmul
                # operands must start at partition 0 on this device, so the
                # odd heads need their own partition-0-based tiles.
                qT_pr = [ppool.tile([128, N_LOC], bf16, tag=f"qP{r}", name=f"qP{r}") for r in range(4)]
                kT_pr = [ppool.tile([128, M], bf16, tag=f"kP{r}", name=f"kP{r}") for r in range(4)]
                qT_od = [ppool.tile([64, N_LOC], bf16, tag=f"qO{r}", name=f"qO{r}") for r in range(4)]
                kT_od = [ppool.tile([64, M], bf16, tag=f"kO{r}", name=f"kO{r}") for r in range(4)]
                v_sb = [ppool.tile([128, DIM], bf16, tag=f"v{cb}", name=f"v{cb}") for cb in range(NCB)]

                def qT(h):
                    return qT_pr[h // 2][0:64, :] if h % 2 == 0 else qT_od[h // 2][:]

                def kT(h):
                    return kT_pr[h // 2][0:64, :] if h % 2 == 0 else kT_od[h // 2][:]

                # single rotating PSUM pool (tag "s": [128,1024] f32, 2
                # banks x 3 bufs) shared by phase-A projections, S^T units
                # and tail transposes/out-proj: avoids the pool-scope WAR
                # that would serialize phase A against the first units.
                with (
                    tc.tile_pool(name="inp", bufs=1) as ipool,
                    tc.tile_pool(name="cntp", bufs=2) as cntp,
                    tc.tile_pool(name="atp", bufs=10) as atp,
                    tc.tile_pool(name="tailp", bufs=2) as tailp,
                    tc.tile_pool(name="psS", bufs=3, space="PSUM") as psS,
                    tc.tile_pool(name="psav", bufs=1, space="PSUM") as psav,
                    tc.tile_pool(name="psden", bufs=1, space="PSUM") as psden,
                ):
                    # ---- phase A: projections (pair-packed) ----
                    xT_sb = [ipool.tile([128, N_LOC], bf16, tag=f"xT{c}", name=f"xT{c}") for c in range(2)]
                    cT_sb = [ipool.tile([128, M], bf16, tag=f"cT{c}", name=f"cT{c}") for c in range(2)]
                    for c in range(2):
                        nc.scalar.dma_start(out=cT_sb[c][:], in_=cT_h[c * 128:(c + 1) * 128, :])
                    for c in range(2):
                        nc.scalar.dma_start(out=xT_sb[c][:], in_=xT_h[c * 128:(c + 1) * 128, :])
                    # wo [512, 256] -> [128, (chunk, 256)] in one 3D-AP DMA
                    nc.scalar.dma_start(
                        out=wo_sb[:].rearrange("p (c j) -> p c j", c=4),
                        in_=wo_h[:].rearrange("(c p) j -> p c j", c=4))
                    nc.scalar.dma_start(out=bo_sb[:], in_=bo_h[:])
                    # phase-A psum: pack TWO projection chunks per s-slot
                    # (separate banks, separate accumulation groups) so PE
                    # gets enough runway to ramp out of the mid p-state
                    pa_state = {"tile": None, "n": 0}

                    def pa_half():
                        if pa_state["tile"] is not None and pa_state["n"] == 1:
                            pa_state["n"] = 2
                            return pa_state["tile"][:, 512:1024]
                        pa_state["tile"] = psS.tile([128, 1024], f32, tag="s", name="pa")
                        pa_state["n"] = 1
                        return pa_state["tile"][:, 0:512]

                    # kT pair r: [128, c] = Wk[:, pair]^T ctx^T; each chunk
                    # is followed by the odd head's partition-0 shuffle DMA
                    def k_proj(r, cc):
                        psk = pa_half()
                        for c in range(2):
                            nc.tensor.matmul(
                                out=psk, lhsT=wk(c, r),
                                rhs=cT_sb[c][:, cc * 512:(cc + 1) * 512],
                                start=(c == 0), stop=(c == 1))
                        dst = kT_pr[r][:, cc * 512:(cc + 1) * 512]
                        if cc % 2 == 0:
                            nc.vector.tensor_copy(out=dst, in_=psk)
                        else:
                            nc.scalar.activation(out=dst, in_=psk, func=Act.Copy)
                        nc.sync.dma_start(
                            out=kT_od[r][:, cc * 512:(cc + 1) * 512],
                            in_=kT_pr[r][64:128, cc * 512:(cc + 1) * 512])

                    def q_proj(r, cc):
                        psq = pa_half()
                        for c in range(2):
                            nc.tensor.matmul(
                                out=psq, lhsT=wq(c, r),
                                rhs=xT_sb[c][:, cc * 512:(cc + 1) * 512],
                                start=(c == 0), stop=(c == 1))
                        nc.scalar.activation(
                            out=qT_pr[r][:, cc * 512:(cc + 1) * 512],
                            in_=psq, func=Act.Identity,
                            bias=bqc_sb[:, r:r + 1])
                        nc.sync.dma_start(
                            out=qT_od[r][:, cc * 512:(cc + 1) * 512],
                            in_=qT_pr[r][64:128, cc * 512:(cc + 1) * 512])

                    # upfront: only what units cb0-3 need; the rest is
                    # deadline-scheduled into tile 0 in packed pairs (each
                    # pair holds one s-slot for < 2 Act periods)
                    for r in range(4):
                        k_proj(r, 0)
                    for r in range(4):
                        q_proj(r, 0)
                    for cb0 in range(4):
                        v_proj(cb0, nc.vector)
                    sched = {
                        0: [("k", 0, 1), ("k", 1, 1)],
                        1: [("k", 2, 1), ("k", 3, 1)],
                        2: [("v", 4), ("v", 5)],
                        3: [("v", 6), ("v", 7)],
                        4: [("k", 0, 2), ("k", 1, 2)],
                        5: [("k", 2, 2), ("k", 3, 2)],
                        6: [("v", 8), ("v", 9)],
                        7: [("v", 10), ("v", 11)],
                        8: [("k", 0, 3), ("k", 1, 3)],
                        9: [("k", 2, 3), ("k", 3, 3)],
                        10: [("v", 12), ("v", 13)],
                        11: [("v", 14), ("v", 15)],
                        12: [("q", 0, 1), ("q", 1, 1)],
                        13: [("q", 2, 1), ("q", 3, 1)],
                    }

                    def v_proj(cb, eng):
                        psv = pa_half()
                        for c in range(2):
                            nc.tensor.matmul(
                                out=psv,
                                lhsT=cT_sb[c][:, cb * 128:(cb + 1) * 128],
                                rhs=wv(c),
                                start=(c == 0), stop=(c == 1))
                        with nc.allow_low_precision(reason="bf16 V"):
                            eng.tensor_copy(out=v_sb[cb][:], in_=psv)

                    # ---- phase B: dense attention ----
                    pend = []  # deferred tail closures [(norm, pe_part)]

                    def mk_tail2(t, avf, rden):
                        ao = tailp.tile([128, DIM], bf16, tag="ao", name=f"ao{t}")

                        def norm():
                            with nc.allow_low_precision(reason="bf16 attention out"):
                                # inner-dim (d) stride-0 broadcast: gpsimd (DVE
                                # lacks it on hw; gpsimd can't read PSUM)
                                nc.gpsimd.tensor_tensor(
                                    out=ao[:].rearrange("p (h d) -> p h d", h=8),
                                    in0=avf[:].rearrange("p (h d) -> p h d", h=8),
                                    in1=rden[:].rearrange("p (h o) -> p h o", o=1
                                                          ).to_broadcast([128, 8, 64]),
                                    op=mybir.AluOpType.mult)

                        aT2 = tailp.tile([128, DIM], bf16, tag="aT2", name=f"aT2{t}")

                        def pe_t():
                            # holds one s-slot for < 1 Act period
                            ptile = psS.tile([128, 1024], f32, tag="s", name=f"pt{t}")
                            pst = ptile[:].bitcast(bf16)[:, 0:DIM]
                            for c in range(4):
                                nc.tensor.transpose(
                                    out=pst[:, c * 128:(c + 1) * 128],
                                    in_=ao[:, c * 128:(c + 1) * 128],
                                    identity=ident[:])
                            nc.vector.tensor_copy(out=aT2[:], in_=pst)

                        def pe_o():
                            ptile = psS.tile([128, 1024], f32, tag="s", name=f"po{t}")
                            op_ps = ptile[:, 0:DIN]
                            for c in range(4):
                                nc.tensor.matmul(
                                    out=op_ps, lhsT=aT2[:, c * 128:(c + 1) * 128],
                                    rhs=wo_sb[:, c * DIN:(c + 1) * DIN],
                                    start=(c == 0), stop=(c == 3))
                            ot = tailp.tile([128, DIN], f32, tag="ot", name=f"ot{t}")
                            nc.vector.tensor_tensor(
                                out=ot[:], in0=op_ps, in1=bo_sb[:],
                                op=mybir.AluOpType.add)
                            nc.sync.dma_start(
                                out=out_h[t * 128:(t + 1) * 128, :], in_=ot[:])

                        return norm, pe_t, pe_o

                    # prefetch the first tile's cnt before the units
                    cnt_tiles = {}
                    cnt_tiles[0] = cntp.tile([128, NCB * 128], bf16, tag="cnt", name="cnt0")
                    nc.sync.dma_start(out=cnt_tiles[0][:], in_=cnt_h[:, 0:2048])

                    from collections import deque
                    av_q = deque()
                    for t in range(NT):
                        cnt_sb = cnt_tiles.pop(t)
                        av_ps = psav.tile([128, DIM], f32, tag="av", name=f"av{t}")
                        den_ps = psden.tile([128, 8], f32, tag="den", name=f"den{t}")
                        for cb in range(NCB):
                            # flush the previous tile's deferred tail work:
                            # Pool-normalize early, PE transposes/out-proj
                            # later (so PE never head-of-line blocks on them)
                            if cb == 1 and pend:
                                pend[-1][0]()
                            if cb == 4 and pend:
                                pend[-1][1]()
                            if cb == 7 and pend:
                                pend.pop()[2]()
                            # deadline-scheduled phase-A remainder rides
                            # the first tiles' slack
                            if t == 0:
                                for item in sched.pop(cb, []):
                                    if item[0] == "k":
                                        k_proj(item[1], item[2])
                                    elif item[0] == "q":
                                        q_proj(item[1], item[2])
                                    else:
                                        v_proj(item[1], nc.vector)
                            # prefetch next tile's cnt mid-tile
                            if cb == 8 and t + 1 < NT:
                                cnt_tiles[t + 1] = cntp.tile(
                                    [128, NCB * 128], bf16, tag="cnt", name=f"cnt{t + 1}")
                                nc.sync.dma_start(
                                    out=cnt_tiles[t + 1][:],
                                    in_=cnt_h[:, (t + 1) * 2048:(t + 2) * 2048])
                            # S^T[c, (h,q)] for this (t, cb)
                            s_ps = psS.tile([128, 1024], f32, tag="s", name=f"s{t}_{cb}")
                            for h in range(8):
                                nc.tensor.matmul(
                                    out=s_ps[:, h * 128:(h + 1) * 128],
                                    lhsT=kT(h)[:, cb * 128:(cb + 1) * 128],
                                    rhs=qT(h)[:, t * 128:(t + 1) * 128],
                                    start=True, stop=True)
                            # A = exp(S^T) (Act), then A *= cnt (DVE)
                            slab = atp.tile([128, 8, 128], bf16, tag="at", name=f"at{t}_{cb}")
                            nc.scalar.activation(
                                out=slab[:].rearrange("p h q -> p (h q)"),
                                in_=s_ps[:], func=Act.Exp)
                            with nc.allow_low_precision(reason="bf16 attention weights"):
                                nc.vector.tensor_tensor(
                                    out=slab[:], in0=slab[:],
                                    in1=cnt_sb[:, cb * 128:(cb + 1) * 128].rearrange(
                                        "p (o q) -> p o q", o=1
                                    ).to_broadcast([128, 8, 128]),
                                    op=mybir.AluOpType.mult)
                            # av[q, (h,d)] += A_h^T V_h ; den[q, h] += A_h^T 1
                            # one PSUM accumulation group per bank: start on
                            # the first matmul, stop on the last. Emission is
                            # deferred 2 units so these matmuls enter the PE
                            # queue with deps resolved (a stalled av chain
                            # fills the 4-deep PE wait queue and blocks the
                            # next unit's S matmuls).
                            def av_mms(cb=cb, slab=slab, av_ps=av_ps, den_ps=den_ps):
                                for h in range(8):
                                    nc.tensor.matmul(
                                        out=av_ps[:, h * 64:(h + 1) * 64],
                                        lhsT=slab[:, h, :],
                                        rhs=v_sb[cb][:, h * 64:(h + 1) * 64],
                                        start=(cb == 0 and h == 0),
                                        stop=(cb == NCB - 1 and h == 7))
                                    nc.tensor.matmul(
                                        out=den_ps[:, h:h + 1],
                                        lhsT=slab[:, h, :],
                                        rhs=ones_sb[:],
                                        start=(cb == 0 and h == 0),
                                        stop=(cb == NCB - 1 and h == 7))
                            av_q.append(av_mms)
                            keep = 2 if cb < 8 else 0
                            while len(av_q) > keep:
                                av_q.popleft()()
                        while av_q:
                            av_q.popleft()()
                        # tail1: free av/den PSUM fast (DVE only)
                        rden = tailp.tile([128, 8], f32, tag="rden", name=f"rden{t}")
                        nc.vector.reciprocal(out=rden[:], in_=den_ps[:])
                        avf = tailp.tile([128, DIM], f32, tag="avf", name=f"avf{t}")
                        nc.vector.tensor_copy(out=avf[:], in_=av_ps[:])
                        pend.append(mk_tail2(t, avf, rden))
                    norm, pe_t, pe_o = pend.pop()
                    norm()
                    pe_t()
                    pe_o()
    nc.compile()
    return nc


def _get_nc():
    if "nc" not in _CACHE:
        _CACHE["nc"] = _build()
    return _CACHE["nc"]


def kernel(**inputs) -> np.ndarray:
    from concourse.bass_utils import run_bass_kernel_spmd
    from ml_dtypes import bfloat16

    x = np.asarray(inputs["x"], dtype=np.float32)
    ctx = np.asarray(inputs["context"], dtype=np.float32)
    idx = np.asarray(inputs["index_pairs"]).astype(np.int64)
    scale = 1.0 / np.sqrt(HD)
    wq = np.asarray(inputs["Wq"], dtype=np.float32) * scale
    wqkv = np.concatenate(
        [wq, np.asarray(inputs["Wk"], dtype=np.float32),
         np.asarray(inputs["Wv"], dtype=np.float32)], axis=1).astype(bfloat16)
    bq = (np.asarray(inputs["bq"], dtype=np.float32) * scale).reshape(4, 128).T
    bq = np.ascontiguousarray(bq).astype(np.float32)  # [128, 4], col r = bq[r*128:(r+1)*128]
    wo = np.asarray(inputs["Wout"], dtype=np.float32).astype(bfloat16)
    bo = np.tile(np.asarray(inputs["bout"], dtype=np.float32).reshape(1, DIN),
                 (128, 1)).astype(np.float32)

    nc = _get_nc()
    in_maps = []
    qrep = np.repeat(np.arange(N_LOC), K)
    for c in range(8):
        b, half = c // 2, c % 2
        xT_c = np.ascontiguousarray(
            x[b, half * N_LOC:(half + 1) * N_LOC, :].T).astype(bfloat16)
        cT_c = np.ascontiguousarray(ctx[b].T).astype(bfloat16)
        idx_c = idx[b, half * N_LOC:(half + 1) * N_LOC, :]  # [1024, 32]
        # neighbor count matrix cnt[c, q], including duplicate multiplicity
        flat = idx_c.reshape(-1) * N_LOC + qrep
        cnt = np.bincount(flat, minlength=M * N_LOC).reshape(M, N_LOC)
        # layout [128 c-part, (t, cb, q)]
        cnt_w = np.ascontiguousarray(
            cnt.reshape(NCB, 128, NT, 128).transpose(1, 2, 0, 3).reshape(128, NT * NCB * 128)
        ).astype(bfloat16)
        in_maps.append({
            "xT": xT_c, "cT": cT_c, "cnt": cnt_w,
            "wqkv": wqkv, "wo": wo, "bq": bq, "bo": bo,
        })
    res = run_bass_kernel_spmd(nc, in_maps, core_ids=list(range(8)))
    out = np.empty((B, N, DIN), dtype=np.float32)
    for c in range(8):
        b, half = c // 2, c % 2
        out[b, half * N_LOC:(half + 1) * N_LOC, :] = res.results[c]["out"]
    return out
